# revision 4
# baseline (speedup 1.0000x reference)
"""Causal self-attention kernel for Trainium2, distributed over 8 NeuronCores.

Problem (full): x[2, 2048, 1024], Wq/Wk/Wv[1024, 16, 64], Wo[16, 64, 1024]
  q/k/v = einsum('bld,dhk->blhk'); scores = q k^T / sqrt(64), causal mask,
  softmax; y = attn @ v; out = einsum('blhk,hkd->bld').

Sharding: core c in 0..7 -> batch b = c // 4, head-group g = c % 4
  (heads [4g, 4g+4)).  Each core computes its batch's partial output
  projection over its 4 heads; the host sums the 4 head-group partials
  per batch (the "all-reduce" of the output projection done host-side
  during unsharding).

Per-core layout strategy (bf16 matmuls, f32 PSUM accumulation):
  - x^T [1024, 2048] resident in SBUF (host pre-transposes).
  - Q^T, K^T computed as [128(d of head-pair), 2, 2048] so scores can be
    computed directly in S^T = [key, query] layout (contraction over d on
    partitions, K=64, two heads row-packed on PE).
  - softmax without max-subtraction (scores are O(10) here so exp is safe):
    additive causal mask on PSUM, exp on ACT with fused 1/8 scale,
    denominator obtained free by appending a ones-column to V in the
    P^T @ [V|1] matmul (row 64 of the PSUM accumulator = row sums).
  - AV in Y^T layout [d, q] (lhsT = [V|1] block, rhs = P^T block), which is
    exactly the layout the output projection needs as lhsT. No transposes
    anywhere in the kernel.
  - causality: key-blocks above the diagonal are skipped entirely; the
    scores/exp/AV column ranges shrink on diagonal blocks.
"""

import sys

sys.path.insert(0, "/opt/trn_rl_repo")

import numpy as np
from contextlib import ExitStack

import concourse.bass as bass
import concourse.mybir as mybir
import concourse.tile as tile
from concourse import bacc

F32 = mybir.dt.float32
BF16 = mybir.dt.bfloat16
AF = mybir.ActivationFunctionType

B, L, D, H, HD = 2, 2048, 1024, 16, 64
NCORES = 8
HG = 4              # heads per core
NG = H // HG        # 4 head-groups
T = HG // 2         # 2 head-pairs per core
P = 128
KC = D // P         # 8 contraction chunks for the projections
QB = 512            # query-range block (moving free dim)
NA = L // QB        # 4 query ranges
NJ = L // P         # 16 key blocks
SCALE = 1.0 / np.sqrt(HD)
NEG = -1.0e9


def _body(ctx: ExitStack, tc: tile.TileContext, xt_d, wq_d, wk_d, wv_d, wo_d, out_d):
    nc = tc.nc

    consts = ctx.enter_context(tc.tile_pool(name="consts", bufs=1))
    pj = ctx.enter_context(tc.tile_pool(name="pj", bufs=2, space="PSUM"))
    ps = ctx.enter_context(tc.tile_pool(name="ps", bufs=2, space="PSUM"))
    py = ctx.enter_context(tc.tile_pool(name="py", bufs=2, space="PSUM"))
    po = ctx.enter_context(tc.tile_pool(name="po", bufs=2, space="PSUM"))
    ptp = ctx.enter_context(tc.tile_pool(name="ptp", bufs=3))
    smp = ctx.enter_context(tc.tile_pool(name="smp", bufs=3))
    obp = ctx.enter_context(tc.tile_pool(name="obp", bufs=3))

    # ---- resident inputs
    xt = consts.tile([P, KC, L], BF16)        # x^T chunks: [p, c, m]
    for c in range(KC):
        nc.gpsimd.dma_start(out=xt[:, c, :], in_=xt_d[c * P:(c + 1) * P, :])
    wq = consts.tile([P, KC, HG * HD], BF16)
    wk = consts.tile([P, KC, HG * HD], BF16)
    wv = consts.tile([P, KC, HG * HD], BF16)
    nc.gpsimd.dma_start(out=wq, in_=wq_d.rearrange("(c p) n -> p c n", p=P))
    nc.gpsimd.dma_start(out=wk, in_=wk_d.rearrange("(c p) n -> p c n", p=P))
    nc.gpsimd.dma_start(out=wv, in_=wv_d.rearrange("(c p) n -> p c n", p=P))
    wo = consts.tile([P, T, D], BF16)
    nc.gpsimd.dma_start(out=wo, in_=wo_d.rearrange("(t p) d -> p t d", p=P))

    # ---- intermediates
    qt = consts.tile([P, T, L], BF16)         # Q^T: [d-of-pair, t, m]
    kt = consts.tile([P, T, L], BF16)
    vsb = consts.tile([P, NJ, HG, HD + 1], BF16)  # [j-in-blk, jb, h, d | ones]
    yt = consts.tile([P, T, L], BF16)         # Y^T (normalized)
    nc.vector.memset(vsb[:, :, :, HD:HD + 1], 1.0)

    # additive causal mask for the diagonal 128x128 strip: keep (0) iff y >= x
    maskadd = consts.tile([P, P], F32)
    nc.gpsimd.memset(maskadd, 0.0)
    nc.gpsimd.affine_select(
        out=maskadd, in_=maskadd, compare_op=mybir.AluOpType.is_ge,
        fill=NEG, base=0, pattern=[[1, P]], channel_multiplier=-1,
    )

    # ---- projections (f32r, contraction over D in 8 chunks of 128)
    for t in range(T):
        for m in range(NA):
            msl = slice(m * QB, (m + 1) * QB)
            pk = pj.tile([P, QB], F32, tag="pj")
            for c in range(KC):
                nc.tensor.matmul(pk, lhsT=wk[:, c, t * P:(t + 1) * P],
                                 rhs=xt[:, c, msl], start=(c == 0), stop=(c == KC - 1))
            nc.any.tensor_copy(out=kt[:, t, msl], in_=pk)
            pq = pj.tile([P, QB], F32, tag="pj")
            for c in range(KC):
                nc.tensor.matmul(pq, lhsT=wq[:, c, t * P:(t + 1) * P],
                                 rhs=xt[:, c, msl], start=(c == 0), stop=(c == KC - 1))
            nc.any.tensor_copy(out=qt[:, t, msl], in_=pq)
    for jb in range(NJ):
        pv = pj.tile([P, HG * HD], F32, tag="pj")
        for c in range(KC):
            nc.tensor.matmul(pv, lhsT=xt[:, c, jb * P:(jb + 1) * P],
                             rhs=wv[:, c, :], start=(c == 0), stop=(c == KC - 1))
        nc.any.tensor_copy(out=vsb[:, jb, :, 0:HD],
                           in_=pv.rearrange("p (h d) -> p h d", h=HG))

    # ---- attention + (interleaved) output projection
    for a in range(NA):
        for t in range(T):
            for u in range(2):
                h = 2 * t + u
                hp = slice(64 * u, 64 * u + 64)
                psy = py.tile([65, QB], F32, tag="py")
                nj = 4 * a + 4
                for j in range(nj):
                    r = j - 4 * a          # >= 0 on diagonal blocks
                    off = 0 if r < 0 else (128 * r if r <= 2 else 256)
                    pss = ps.tile([P, QB], F32, tag="ps")
                    nc.tensor.matmul(
                        pss[:, off:QB],
                        lhsT=kt[hp, t, j * P:(j + 1) * P],
                        rhs=qt[hp, t, a * QB + off:(a + 1) * QB],
                        start=True, stop=True,
                    )
                    if r >= 0:
                        nc.vector.tensor_add(pss[:, 128 * r:128 * (r + 1)],
                                             pss[:, 128 * r:128 * (r + 1)], maskadd)
                    pt = ptp.tile([P, QB], BF16, tag="pt")
                    eoff = off if r != 3 else 384
                    nc.scalar.activation(pt[:, eoff:QB], pss[:, eoff:QB],
                                         AF.Exp, scale=float(SCALE))
                    if r == 3:
                        nc.vector.memset(pt[:, 256:384], 0.0)
                    nc.tensor.matmul(
                        psy[:, off:QB],
                        lhsT=vsb[:, j, h, :],
                        rhs=pt[:, off:QB],
                        start=(j == 0), stop=(j == nj - 1),
                    )
                # normalize: y^T = Y^T / denominator (row 64 of psy)
                rec = smp.tile([1, QB], F32, tag="rec")
                nc.vector.reciprocal(rec, psy[64:65, :])
                den = smp.tile([64, QB], F32, tag="den")
                nc.gpsimd.partition_broadcast(den, rec)
                nc.vector.tensor_mul(yt[hp, t, a * QB:(a + 1) * QB],
                                     psy[0:64, :], den)
        # output projection for the 4 finished m-blocks of this a-range
        for mi in range(4):
            m = 4 * a + mi
            for db in range(2):
                dsl = slice(db * QB, (db + 1) * QB)
                pso = po.tile([P, QB], F32, tag="po")
                for t in range(T):
                    nc.tensor.matmul(
                        pso,
                        lhsT=yt[:, t, m * P:(m + 1) * P],
                        rhs=wo[:, t, dsl],
                        start=(t == 0), stop=(t == T - 1),
                    )
                ob = obp.tile([P, QB], F32, tag="ob")
                nc.any.tensor_copy(out=ob, in_=pso)
                nc.sync.dma_start(out=out_d[m * P:(m + 1) * P, dsl], in_=ob)


_NC_CACHE = None


def _build_nc():
    global _NC_CACHE
    if _NC_CACHE is not None:
        return _NC_CACHE
    nc = bacc.Bacc("TRN2", target_bir_lowering=False, debug=False,
                   enable_asserts=False)
    xt_d = nc.dram_tensor("xt", [D, L], F32, kind="ExternalInput")
    wq_d = nc.dram_tensor("wq", [D, HG * HD], F32, kind="ExternalInput")
    wk_d = nc.dram_tensor("wk", [D, HG * HD], F32, kind="ExternalInput")
    wv_d = nc.dram_tensor("wv", [D, HG * HD], F32, kind="ExternalInput")
    wo_d = nc.dram_tensor("wo", [HG * HD, D], F32, kind="ExternalInput")
    out_d = nc.dram_tensor("out", [L, D], F32, kind="ExternalOutput")
    with tile.TileContext(nc) as tc, ExitStack() as ctx:
        _body(ctx, tc, xt_d.ap(), wq_d.ap(), wk_d.ap(), wv_d.ap(), wo_d.ap(),
              out_d.ap())
    nc.compile()
    _NC_CACHE = nc
    return nc


def _shard_inputs(x_bld, Wq, Wk, Wv, Wo):
    x_bld = np.asarray(x_bld, dtype=np.float32)
    Wq = np.asarray(Wq, dtype=np.float32)
    Wk = np.asarray(Wk, dtype=np.float32)
    Wv = np.asarray(Wv, dtype=np.float32)
    Wo = np.asarray(Wo, dtype=np.float32)
    in_maps = []
    for c in range(NCORES):
        b, g = divmod(c, NG)
        hsl = slice(g * HG, (g + 1) * HG)
        in_maps.append({
            "xt": np.ascontiguousarray(x_bld[b].T),                      # [D, L]
            "wq": np.ascontiguousarray(Wq[:, hsl, :].reshape(D, HG * HD)),
            "wk": np.ascontiguousarray(Wk[:, hsl, :].reshape(D, HG * HD)),
            "wv": np.ascontiguousarray(Wv[:, hsl, :].reshape(D, HG * HD)),
            "wo": np.ascontiguousarray(Wo[hsl].reshape(HG * HD, D)),
        })
    return in_maps


def _combine(outs):
    y = np.zeros((B, L, D), dtype=np.float32)
    for c in range(NCORES):
        y[c // NG] += outs[c]
    return y


LAST_RESULT = None


def kernel(x_bld, Wq, Wk, Wv, Wo):
    global LAST_RESULT
    from concourse.bass_utils import run_bass_kernel_spmd
    nc = _build_nc()
    in_maps = _shard_inputs(x_bld, Wq, Wk, Wv, Wo)
    res = run_bass_kernel_spmd(nc, in_maps, core_ids=list(range(NCORES)))
    LAST_RESULT = res
    return _combine([res.results[c]["out"] for c in range(NCORES)])


# revision 7
# speedup vs baseline: 1.1173x; 1.1173x over previous
"""Causal self-attention kernel for Trainium2, distributed over 8 NeuronCores.

Problem (full): x[2, 2048, 1024], Wq/Wk/Wv[1024, 16, 64], Wo[16, 64, 1024]
  q/k/v = einsum('bld,dhk->blhk'); scores = q k^T / sqrt(64), causal mask,
  softmax; y = attn @ v; out = einsum('blhk,hkd->bld').

Sharding: core c in 0..7 -> batch b = c // 4, head-group g = c % 4
  (heads [4g, 4g+4)).  Each core computes its batch's partial output
  projection over its 4 heads; the host sums the 4 head-group partials
  per batch (the "all-reduce" of the output projection done host-side
  during unsharding).

Per-core layout strategy (bf16 matmuls, f32 PSUM accumulation):
  - x^T [1024, 2048] resident in SBUF (host pre-transposes).
  - Q^T, K^T computed as [128(d of head-pair), 2, 2048] so scores can be
    computed directly in S^T = [key, query] layout (contraction over d on
    partitions, K=64, two heads row-packed on PE).
  - softmax without max-subtraction (scores are O(10) here so exp is safe):
    additive causal mask on PSUM, exp on ACT with fused 1/8 scale,
    denominator obtained free by appending a ones-column to V in the
    P^T @ [V|1] matmul (row 64 of the PSUM accumulator = row sums).
  - AV in Y^T layout [d, q] (lhsT = [V|1] block, rhs = P^T block), which is
    exactly the layout the output projection needs as lhsT. No transposes
    anywhere in the kernel.
  - causality: key-blocks above the diagonal are skipped entirely; the
    scores/exp/AV column ranges shrink on diagonal blocks.
"""

import sys

sys.path.insert(0, "/opt/trn_rl_repo")

import ml_dtypes
import numpy as np
from contextlib import ExitStack

import concourse.bass as bass
import concourse.mybir as mybir
import concourse.tile as tile
from concourse import bacc

F32 = mybir.dt.float32
BF16 = mybir.dt.bfloat16
AF = mybir.ActivationFunctionType

B, L, D, H, HD = 2, 2048, 1024, 16, 64
NCORES = 8
HG = 4              # heads per core
NG = H // HG        # 4 head-groups
T = HG // 2         # 2 head-pairs per core
P = 128
KC = D // P         # 8 contraction chunks for the projections
QB = 512            # query-range block (moving free dim)
NA = L // QB        # 4 query ranges
NJ = L // P         # 16 key blocks
SCALE = 1.0 / np.sqrt(HD)
NEG = -1.0e9


def _body(ctx: ExitStack, tc: tile.TileContext, xt_d, wq_d, wk_d, wv_d, wo_d, out_d):
    nc = tc.nc

    consts = ctx.enter_context(tc.tile_pool(name="consts", bufs=1))
    pj = ctx.enter_context(tc.tile_pool(name="pj", bufs=2, space="PSUM"))
    ps = ctx.enter_context(tc.tile_pool(name="ps", bufs=2, space="PSUM"))
    py = ctx.enter_context(tc.tile_pool(name="py", bufs=1, space="PSUM"))
    po = pj
    ptp = ctx.enter_context(tc.tile_pool(name="ptp", bufs=3))
    smp = ctx.enter_context(tc.tile_pool(name="smp", bufs=3))
    obp = ctx.enter_context(tc.tile_pool(name="obp", bufs=3))

    # ---- resident inputs
    xt = consts.tile([P, KC, L], BF16)        # x^T chunks: [p, c, m]
    for c in range(KC):
        nc.sync.dma_start(out=xt[:, c, :], in_=xt_d[c * P:(c + 1) * P, :])
    wq = consts.tile([P, KC, HG * HD], BF16)
    wk = consts.tile([P, KC, HG * HD], BF16)
    wv = consts.tile([P, KC, HG * HD], BF16)
    nc.sync.dma_start(out=wq, in_=wq_d.rearrange("(c p) n -> p c n", p=P))
    nc.sync.dma_start(out=wk, in_=wk_d.rearrange("(c p) n -> p c n", p=P))
    nc.sync.dma_start(out=wv, in_=wv_d.rearrange("(c p) n -> p c n", p=P))
    wo = consts.tile([P, T, D], BF16)
    nc.sync.dma_start(out=wo, in_=wo_d.rearrange("(t p) d -> p t d", p=P))

    # ---- intermediates
    qt = consts.tile([P, T, L], BF16)         # Q^T: [d-of-pair, t, m]
    kt = consts.tile([P, T, L], BF16)
    vsb = consts.tile([P, NJ, HG, HD + 1], BF16)  # [j-in-blk, jb, h, d | ones]
    yt = consts.tile([P, T, L], BF16)         # Y^T (normalized)
    nc.vector.memset(vsb[:, :, :, HD:HD + 1], 1.0)

    # additive causal mask for the diagonal 128x128 strip: keep (0) iff y >= x
    maskadd = consts.tile([P, P], F32)
    nc.gpsimd.memset(maskadd, 0.0)
    nc.gpsimd.affine_select(
        out=maskadd, in_=maskadd, compare_op=mybir.AluOpType.is_ge,
        fill=NEG, base=0, pattern=[[1, P]], channel_multiplier=-1,
    )

    # ---- projections (f32r, contraction over D in 8 chunks of 128)
    for t in range(T):
        for m in range(NA):
            msl = slice(m * QB, (m + 1) * QB)
            pk = pj.tile([P, QB], F32, tag="pj")
            for c in range(KC):
                nc.tensor.matmul(pk, lhsT=wk[:, c, t * P:(t + 1) * P],
                                 rhs=xt[:, c, msl], start=(c == 0), stop=(c == KC - 1))
            nc.any.tensor_copy(out=kt[:, t, msl], in_=pk)
            pq = pj.tile([P, QB], F32, tag="pj")
            for c in range(KC):
                nc.tensor.matmul(pq, lhsT=wq[:, c, t * P:(t + 1) * P],
                                 rhs=xt[:, c, msl], start=(c == 0), stop=(c == KC - 1))
            nc.any.tensor_copy(out=qt[:, t, msl], in_=pq)
    for jb in range(NJ):
        pv = pj.tile([P, HG * HD], F32, tag="pj")
        for c in range(KC):
            nc.tensor.matmul(pv, lhsT=xt[:, c, jb * P:(jb + 1) * P],
                             rhs=wv[:, c, :], start=(c == 0), stop=(c == KC - 1))
        nc.any.tensor_copy(out=vsb[:, jb, :, 0:HD],
                           in_=pv.rearrange("p (h d) -> p h d", h=HG))

    # ---- attention + (interleaved) output projection
    for a in range(NA):
        for t in range(T):
            psys = [py.tile([65, QB], F32, tag=f"py{u}", name=f"psy{u}") for u in range(2)]
            nj = 4 * a + 4
            for j in range(nj):
                r = j - 4 * a          # >= 0 on diagonal blocks
                off = 0 if r < 0 else (128 * r if r <= 2 else 256)
                psss = []
                # scores for both heads back-to-back: K=64 row groups 0-1 and
                # 2-3 -> the PE runs them concurrently (separate PSUM tiles)
                for u in range(2):
                    hp = slice(64 * u, 64 * u + 64)
                    pss = ps.tile([P, QB], F32, tag=f"ps{u}")
                    nc.tensor.matmul(
                        pss[:, off:QB],
                        lhsT=kt[hp, t, j * P:(j + 1) * P],
                        rhs=qt[hp, t, a * QB + off:(a + 1) * QB],
                        start=True, stop=True,
                    )
                    psss.append(pss)
                for u in range(2):
                    pss = psss[u]
                    if r >= 0:
                        nc.vector.tensor_add(pss[:, 128 * r:128 * (r + 1)],
                                             pss[:, 128 * r:128 * (r + 1)], maskadd)
                    pt = ptp.tile([P, QB], BF16, tag=f"pt{u}")
                    eoff = off if r != 3 else 384
                    nc.scalar.activation(pt[:, eoff:QB], pss[:, eoff:QB],
                                         AF.Exp, scale=float(SCALE))
                    if r == 3:
                        nc.vector.memset(pt[:, 256:384], 0.0)
                    nc.tensor.matmul(
                        psys[u][:, off:QB],
                        lhsT=vsb[:, j, 2 * t + u, :],
                        rhs=pt[:, off:QB],
                        start=(j == 0), stop=(j == nj - 1),
                    )
            # normalize: y^T = Y^T / denominator (row 64 of psy)
            for u in range(2):
                hp = slice(64 * u, 64 * u + 64)
                rec = smp.tile([1, QB], F32, tag="rec")
                nc.vector.reciprocal(rec, psys[u][64:65, :])
                den = smp.tile([64, QB], F32, tag="den")
                nc.gpsimd.partition_broadcast(den, rec)
                nc.vector.tensor_mul(yt[hp, t, a * QB:(a + 1) * QB],
                                     psys[u][0:64, :], den)
        # output projection for the 4 finished m-blocks of this a-range
        for mi in range(4):
            m = 4 * a + mi
            for db in range(2):
                dsl = slice(db * QB, (db + 1) * QB)
                pso = po.tile([P, QB], F32, tag="pj")
                for t in range(T):
                    nc.tensor.matmul(
                        pso,
                        lhsT=yt[:, t, m * P:(m + 1) * P],
                        rhs=wo[:, t, dsl],
                        start=(t == 0), stop=(t == T - 1),
                    )
                ob = obp.tile([P, QB], F32, tag="ob")
                nc.any.tensor_copy(out=ob, in_=pso)
                nc.sync.dma_start(out=out_d[m * P:(m + 1) * P, dsl], in_=ob)


_NC_CACHE = None


def _build_nc():
    global _NC_CACHE
    if _NC_CACHE is not None:
        return _NC_CACHE
    nc = bacc.Bacc("TRN2", target_bir_lowering=False, debug=False,
                   enable_asserts=False)
    xt_d = nc.dram_tensor("xt", [D, L], BF16, kind="ExternalInput")
    wq_d = nc.dram_tensor("wq", [D, HG * HD], BF16, kind="ExternalInput")
    wk_d = nc.dram_tensor("wk", [D, HG * HD], BF16, kind="ExternalInput")
    wv_d = nc.dram_tensor("wv", [D, HG * HD], BF16, kind="ExternalInput")
    wo_d = nc.dram_tensor("wo", [HG * HD, D], BF16, kind="ExternalInput")
    out_d = nc.dram_tensor("out", [L, D], F32, kind="ExternalOutput")
    with tile.TileContext(nc) as tc, ExitStack() as ctx:
        _body(ctx, tc, xt_d.ap(), wq_d.ap(), wk_d.ap(), wv_d.ap(), wo_d.ap(),
              out_d.ap())
    nc.compile()
    _NC_CACHE = nc
    return nc


def _shard_inputs(x_bld, Wq, Wk, Wv, Wo):
    x_bld = np.asarray(x_bld, dtype=np.float32)
    Wq = np.asarray(Wq, dtype=np.float32)
    Wk = np.asarray(Wk, dtype=np.float32)
    Wv = np.asarray(Wv, dtype=np.float32)
    Wo = np.asarray(Wo, dtype=np.float32)
    in_maps = []
    for c in range(NCORES):
        b, g = divmod(c, NG)
        hsl = slice(g * HG, (g + 1) * HG)
        bf = ml_dtypes.bfloat16
        in_maps.append({
            "xt": np.ascontiguousarray(x_bld[b].T.astype(bf)),           # [D, L]
            "wq": np.ascontiguousarray(Wq[:, hsl, :].reshape(D, HG * HD).astype(bf)),
            "wk": np.ascontiguousarray(Wk[:, hsl, :].reshape(D, HG * HD).astype(bf)),
            "wv": np.ascontiguousarray(Wv[:, hsl, :].reshape(D, HG * HD).astype(bf)),
            "wo": np.ascontiguousarray(Wo[hsl].reshape(HG * HD, D).astype(bf)),
        })
    return in_maps


def _combine(outs):
    y = np.zeros((B, L, D), dtype=np.float32)
    for c in range(NCORES):
        y[c // NG] += outs[c]
    return y


LAST_RESULT = None


def kernel(x_bld, Wq, Wk, Wv, Wo):
    global LAST_RESULT
    from concourse.bass_utils import run_bass_kernel_spmd
    nc = _build_nc()
    in_maps = _shard_inputs(x_bld, Wq, Wk, Wv, Wo)
    res = run_bass_kernel_spmd(nc, in_maps, core_ids=list(range(NCORES)))
    LAST_RESULT = res
    return _combine([res.results[c]["out"] for c in range(NCORES)])
